# revision 55
# baseline (speedup 1.0000x reference)
"""Trainium2 Bass kernel for nn_DiffusionModel_56822417326086.

Causal multi-head self-attention block:
    qkv = x @ w_qkv ; split into 8 heads of 64
    e = (q @ k^T) * DH^-0.5 ; causal + key-padding mask ; a = softmax(e)
    o = a @ v ; y = o @ w_out + b_out ; y *= m

Sharding (8 cores, zero collectives):
    core c -> batch b = c // 2, head-quad q = c % 2 (heads 4q..4q+3).
    Each core computes q/k/v for its 4 heads over its whole batch, full
    causal attention for those heads, and the partial output projection
    y_partial = o[heads] @ w_out[head rows].  Host sums the two partials
    per batch (linear unshard), adds b_out, applies the query-side mask.

v2 layout/scheduling notes (on top of the v1 design):
  - every input is host-packed so it loads as ONE partition-contiguous
    DMA (x as 4 column-chunks so the first QKV group starts early).
  - diagonal key blocks only compute/exp/accumulate the live query
    range (causal narrowing): scores, exp, mask-mul and A@V all shrink.
  - each (pair, qc) chunk's normalization + output projection is
    emitted inline so it overlaps the remaining attention; only the
    last chunk's chain trails the kernel.
  - softmax reciprocal uses reciprocal_approx_fast (~18 bits, 5x
    faster than the exact DVE reciprocal).
  - scores are computed TRANSPOSED: s[key, query] so the A@V
    contraction (over keys) has keys on the partition dim; softmax
    denominators come free as a 65th "ones" column of V; no
    max-subtraction (scores are O(1), exp is safe); matmuls are f32r /
    bf16; per-head operands sit at partition base 0/64 via the
    row-tiled PE array (tile_position).
"""

import numpy as np
import ml_dtypes
from contextlib import ExitStack

B, T, D, H = 4, 2048, 512, 8
DH = D // H
SCALE = DH ** -0.5
QC = 512           # query-chunk (free dim of score matmuls)
NQC = T // QC      # 4
KB = 128           # key-block (partition dim of score tiles)

_CACHE = {}


def _build_program():
    import concourse.mybir as mybir
    import concourse.tile as tile
    from concourse import bacc

    f32 = mybir.dt.float32
    f32r = mybir.dt.float32r
    bf16 = mybir.dt.bfloat16
    Exp = mybir.ActivationFunctionType.Exp

    nc = bacc.Bacc("TRN2", target_bir_lowering=False, debug=False)

    xp_d = nc.dram_tensor("xp", [128, 4, T], bf16, kind="ExternalInput").ap()
    wq_d = nc.dram_tensor("wqp", [128, 2, 4, 128], bf16, kind="ExternalInput").ap()
    wk_d = nc.dram_tensor("wkp", [128, 2, 4, 128], bf16, kind="ExternalInput").ap()
    wv_d = nc.dram_tensor("wvp", [128, 4, 256], bf16, kind="ExternalInput").ap()
    wo_d = nc.dram_tensor("wop", [128, 2, D], f32r, kind="ExternalInput").ap()
    dm_d = nc.dram_tensor("dmp", [128, 4, 2, 512], bf16, kind="ExternalInput").ap()
    mk_d = nc.dram_tensor("mkp", [128, 16], f32, kind="ExternalInput").ap()
    y_d = nc.dram_tensor("y", [T, D], f32, kind="ExternalOutput").ap()

    with tile.TileContext(nc) as tc, ExitStack() as ctx:
        consts = ctx.enter_context(tc.tile_pool(name="consts", bufs=1))
        work = ctx.enter_context(tc.tile_pool(name="work", bufs=2))
        ps_s = ctx.enter_context(tc.tile_pool(name="pss", bufs=2, space="PSUM"))
        ps_o = ctx.enter_context(tc.tile_pool(name="pso", bufs=1, space="PSUM"))
        ps_y = ctx.enter_context(tc.tile_pool(name="psy", bufs=2, space="PSUM"))

        # ---- persistent tiles ----------------------------------------------
        # packed q^T/k^T: partitions 0-63 = head A of pair, 64-127 = head B
        qT2 = consts.tile([128, 2, T], f32r)
        kT2 = consts.tile([128, 2, T], f32r)
        vsb = consts.tile([128, 16, 4, 65], bf16)
        wo = consts.tile([128, 2, D], f32r)
        mk = consts.tile([128, 16], f32)
        dm2 = consts.tile([128, 4, 2, 512], bf16)
        ones41 = consts.tile([128, 4, 1], f32)
        oUA = consts.tile([65, 2, T], f32)
        oUB = consts.tile([65, 2, T], f32)
        # pair-0 sums partition-parallel (rows 0-3, one batched recip, rec
        # row DMA'd to partition 0); pair-1 reciprocals computed in place at
        # partition 64 (straight from the oU sums row — legal DVE base) and
        # fed to the bc matmul via tile_position=(64, 0), no DMA hops.
        sums_stage = consts.tile([4, 1024], f32)    # row = qc (pair 0)
        recips = consts.tile([4, 1024], f32)
        recb = consts.tile([4, 1024], bf16)
        sums1 = consts.tile([1, 4, 1024], f32)      # [0, qc, :] (pair 1)
        recips1 = consts.tile([1, 4, 1024], f32)
        recb1 = consts.tile([1, 4, 1024], bf16)
        ones64 = consts.tile([1, 64], f32)
        ones64b = consts.tile([1, 64], bf16)
        oTn2 = consts.tile([128, 2, T], f32r)
        dummy = consts.tile([1, 1], f32)

        # ---- input DMAs (one big descriptor set each) + PE warmup ----------
        xT = consts.tile([128, 4, T], bf16)
        wq = consts.tile([128, 2, 4, 128], bf16)
        wk = consts.tile([128, 2, 4, 128], bf16)
        wv = consts.tile([128, 4, 256], bf16)

        # first QKV group needs wq/wk + x cols 0:512 -> issue those first on
        # the two fast HWDGE engines; the rest trickles in on gpsimd
        nc.scalar.dma_start(xT[:, :, 0:512], xp_d[:, :, 0:512])
        nc.sync.dma_start(wq[:], wq_d[:])
        nc.sync.dma_start(wk[:], wk_d[:])
        nc.scalar.dma_start(xT[:, :, 512:1024], xp_d[:, :, 512:1024])
        nc.sync.dma_start(xT[:, :, 1024:1536], xp_d[:, :, 1024:1536])
        nc.scalar.dma_start(xT[:, :, 1536:2048], xp_d[:, :, 1536:2048])
        nc.gpsimd.dma_start(wv[:], wv_d[:])
        nc.gpsimd.dma_start(wo[:], wo_d[:])
        nc.gpsimd.dma_start(mk[:], mk_d[:])
        nc.gpsimd.dma_start(dm2[:], dm_d[:])

        nc.vector.memset(ones41[:], 1.0)
        nc.vector.memset(ones64[:], 1.0)
        nc.vector.memset(dummy[:], 0.0)
        nc.vector.tensor_copy(ones64b[:], ones64[:])
        # preload the Exp table while DMAs are in flight
        nc.scalar.activation(dummy[:], dummy[:], Exp)
        warm = consts.tile([1, 512], bf16)
        nc.vector.memset(warm[:], 1.0)
        for _ in range(6):
            wps = ps_y.tile([64, 512], f32, tag="py")
            nc.tensor.matmul(wps[:], ones64b[:], warm[:], start=True, stop=True)

        # ---- q/k projection (emitted interleaved with pair-0 attention) ----
        def qk_group(nc, p, rc4):
            sl = slice(rc4 * 512, (rc4 + 1) * 512)
            psq = ps_s.tile([128, 512], f32, tag="s")
            psk = ps_y.tile([128, 512], f32, tag="py")
            for kc in range(4):
                nc.tensor.matmul(psq[:], wq[:, p, kc, :], xT[:, kc, sl],
                                 start=kc == 0, stop=kc == 3)
                nc.tensor.matmul(psk[:], wk[:, p, kc, :], xT[:, kc, sl],
                                 start=kc == 0, stop=kc == 3)
            nc.vector.tensor_copy(qT2[:, p, sl], psq[:])
            nc.vector.tensor_copy(kT2[:, p, sl], psk[:])

        def v_group(nc, rc):
            # ps_y home so mid-chunk injection never waits on the oA/oB
            # accumulators of the chunk in flight
            psv = ps_y.tile([128, 4, 64], f32, tag="py")
            for kc in range(4):
                nc.tensor.matmul(psv[:], xT[:, kc, rc * 128:(rc + 1) * 128],
                                 wv[:, kc, :], start=kc == 0, stop=kc == 3)
            nc.vector.tensor_scalar_mul(vsb[:, rc, :, 0:64], psv[:],
                                        mk[:, rc:rc + 1])
            nc.vector.tensor_scalar_mul(vsb[:, rc, :, 64:65], ones41[:],
                                        mk[:, rc:rc + 1])

        # ---- attention with inline postludes --------------------------------
        def attn_chunk(nc, exp_pool, p, qc, inject=None):
            """Score+exp+mask+AV loop for one (pair, query-chunk).

            inject: {kb: [thunk, ...]} -- filler work (next chunk's qk/v
            projection groups) emitted inside the loop so the exp stream
            never starves at chunk boundaries."""
            nkb = 4 * (qc + 1)
            q0 = qc * QC
            oA = ps_o.tile([128, 512], f32, tag="oA")
            oB = ps_o.tile([128, 512], f32, tag="oB")
            avq = []

            def emit_av(item):
                kb, off, ex = item
                nc.tensor.matmul(oA[0:65, off:512], vsb[:, kb, 2 * p, :],
                                 ex[:, 0, off:512],
                                 start=kb == 0, stop=kb == nkb - 1)
                nc.tensor.matmul(oB[0:65, off:512], vsb[:, kb, 2 * p + 1, :],
                                 ex[:, 1, off:512],
                                 start=kb == 0, stop=kb == nkb - 1)

            for kb in range(nkb):
                kbrel = kb - (nkb - 4)
                off = max(0, kbrel) * KB
                ksl = slice(kb * KB, (kb + 1) * KB)
                qsl = slice(q0 + off, q0 + QC)
                sps = ps_s.tile([128, 2, 512], f32, tag="s")
                # row-tiled pair: K=64 each, concurrent in the array;
                # outputs land in DIFFERENT PSUM banks (same-bank
                # dual-write faults the exec unit)
                nc.tensor.matmul(sps[:, 0, off:512], kT2[0:64, p, ksl],
                                 qT2[0:64, p, qsl], start=True, stop=True,
                                 tile_position=(0, 0))
                nc.tensor.matmul(sps[:, 1, off:512], kT2[64:128, p, ksl],
                                 qT2[64:128, p, qsl], start=True, stop=True,
                                 tile_position=(64, 0))
                ex = exp_pool.tile([128, 2, 512], bf16, tag="exp")
                nc.scalar.activation(ex[:, :, off:512], sps[:, :, off:512],
                                     Exp, scale=SCALE)
                if kbrel >= 0:
                    nc.vector.tensor_mul(ex[:, :, off:512], ex[:, :, off:512],
                                         dm2[:, kbrel, :, off:512])
                avq.append((kb, off, ex))
                if len(avq) > 2:
                    emit_av(avq.pop(0))
                if inject and kb in inject:
                    for thunk in inject[kb]:
                        thunk()
            while avq:
                emit_av(avq.pop(0))
            return oA, oB

        def copies_and_sums(nc, p, qc, oA, oB, c0=0, cl=512, cast=True):
            qsl = slice(qc * QC + c0, qc * QC + c0 + cl)
            nc.vector.tensor_copy(oUA[0:65, p, qsl], oA[0:65, c0:c0 + cl])
            nc.vector.tensor_copy(oUB[0:65, p, qsl], oB[0:65, c0:c0 + cl])
            if p == 0:
                nc.gpsimd.dma_start(sums_stage[qc:qc + 1, 0:512],
                                    oUA[64:65, 0, qsl])
                nc.gpsimd.dma_start(sums_stage[qc:qc + 1, 512:1024],
                                    oUB[64:65, 0, qsl])
            else:
                sA = slice(2 * c0, 2 * c0 + cl)
                sB = slice(2 * c0 + cl, 2 * c0 + 2 * cl)
                nc.sync.dma_start(sums1[0:1, qc, sA], oUA[64:65, 1, qsl])
                nc.sync.dma_start(sums1[0:1, qc, sB], oUB[64:65, 1, qsl])
                sAB = slice(2 * c0, 2 * c0 + 2 * cl)
                nc.vector.reciprocal_approx_fast(recips1[0:1, qc, sAB],
                                                 sums1[0:1, qc, sAB])
                if cast:
                    nc.vector.tensor_copy(recb1[0:1, qc, sAB],
                                          recips1[0:1, qc, sAB])

        def norm_chain(nc, p, qc, c0=0, cl=512, raw_rec=False):
            """rec broadcast + normalize into oTn2 for one (pair, qc)."""
            qsl = slice(qc * QC + c0, qc * QC + c0 + cl)
            ones = ones64b[:]
            if p == 0:
                rec = work.tile([1, 1024], bf16, tag="rec")
                nc.sync.dma_start(rec[:], recb[qc:qc + 1, :])
                recA, recB = rec[0:1, 0:512], rec[0:1, 512:1024]
            elif raw_rec:
                # skip the bf16 cast on the tail chain; f32 moving operand
                recA = recips1[0:1, qc, 2 * c0:2 * c0 + cl]
                recB = recips1[0:1, qc, 2 * c0 + cl:2 * c0 + 2 * cl]
                ones = ones64[:]
            else:
                recA = recb1[0:1, qc, 2 * c0:2 * c0 + cl]
                recB = recb1[0:1, qc, 2 * c0 + cl:2 * c0 + 2 * cl]
            bcA = ps_y.tile([64, 512], f32, tag="py")
            nc.tensor.matmul(bcA[0:64, 0:cl], ones, recA,
                             start=True, stop=True)
            nc.vector.tensor_mul(oTn2[0:64, p, qsl], oUA[0:64, p, qsl],
                                 bcA[0:64, 0:cl])
            bcB = ps_y.tile([64, 512], f32, tag="py")
            nc.tensor.matmul(bcB[0:64, 0:cl], ones, recB,
                             start=True, stop=True)
            scrB = work.tile([64, 512], f32r, tag="scrB")
            nc.vector.tensor_mul(scrB[0:64, 0:cl], oUB[0:64, p, qsl],
                                 bcB[0:64, 0:cl])
            # partition shift 0-63 -> 64-127 (DVE lanes are partition-locked)
            nc.sync.dma_start(oTn2[64:128, p, qsl], scrB[0:64, 0:cl])

        def out_proj(nc, rcs, tail=False, act_copy=False):
            # gpsimd (Pool) DGE has multi-us start latency -- fine for hidden
            # mid-kernel stores, fatal for the tail ones.  Tail y stores go
            # on scalar so the sync ring stays clear for the shift DMAs.
            engs = [nc.scalar, nc.scalar] if tail else \
                [nc.sync, nc.gpsimd, nc.sync, nc.gpsimd]
            for i, rc in enumerate(rcs):
                rsl = slice(rc * 128, (rc + 1) * 128)
                psy = ps_y.tile([128, 512], f32, tag="py")
                for p in range(2):
                    nc.tensor.matmul(psy[:], oTn2[:, p, rsl], wo[:, p, :],
                                     start=p == 0, stop=p == 1)
                yt = work.tile([128, 512], f32, tag="ysb", bufs=4)
                if act_copy and i == 0:
                    nc.scalar.copy(yt[:], psy[:])
                else:
                    nc.vector.tensor_copy(yt[:], psy[:])
                engs[i].dma_start(y_d[rsl, :], yt[:])

        with tc.tile_pool(name="exp", bufs=6) as exp_pool:
            # pair 0: qk/v-projection groups interleaved chunk-by-chunk so
            # the exp stream (the attention pacer) starts as early as
            # possible; later chunks' qk/v groups are injected INSIDE the
            # kb loop (one group per iteration) so ACT never starves at
            # chunk boundaries
            for qc in range(NQC):
                qk_group(nc, 0, qc)
                if qc == 0:
                    for rc in range(4):
                        v_group(nc, rc)
                inj = {}
                if qc < NQC - 1:
                    for j in range(4):
                        rc = 4 * (qc + 1) + j
                        inj.setdefault(j, []).append(
                            lambda rc=rc: v_group(nc, rc))
                if qc >= 1:
                    inj.setdefault(4, []).append(
                        lambda p1qc=qc - 1: qk_group(nc, 1, p1qc))
                oA, oB = attn_chunk(nc, exp_pool, 0, qc, inject=inj)
                copies_and_sums(nc, 0, qc, oA, oB)

            # pair 1: attention with pair-0 normalization interleaved and
            # pair-1 postludes inline (chunk qc's full output emitted as
            # soon as its attention is done)
            for qc in range(NQC):
                inj = {1: [lambda: qk_group(nc, 1, 3)]} if qc == 0 else None
                oA, oB = attn_chunk(nc, exp_pool, 1, qc, inject=inj)
                if qc == 0:
                    nc.vector.reciprocal_approx_fast(recips[0:4, :],
                                                     sums_stage[0:4, :])
                    nc.vector.tensor_copy(recb[0:4, :], recips[0:4, :])
                if qc < NQC - 1:
                    copies_and_sums(nc, 1, qc, oA, oB)
                    norm_chain(nc, 0, qc)
                    norm_chain(nc, 1, qc)
                    out_proj(nc, range(4 * qc, 4 * qc + 4))
                    if qc == NQC - 2:
                        # pair-0's last-chunk normalization has no late deps;
                        # run it under the last chunk's attention
                        norm_chain(nc, 0, NQC - 1)
                else:
                    # last chunk: postlude in two pipelined 256-col halves so
                    # the tail chain shortens
                    copies_and_sums(nc, 1, qc, oA, oB, 0, 256, cast=False)
                    copies_and_sums(nc, 1, qc, oA, oB, 256, 256, cast=False)
                    norm_chain(nc, 1, qc, 0, 256, raw_rec=True)
                    out_proj(nc, [4 * qc, 4 * qc + 1], tail=True)
                    norm_chain(nc, 1, qc, 256, 256, raw_rec=True)
                    out_proj(nc, [4 * qc + 2, 4 * qc + 3], tail=True,
                             act_copy=True)

    nc.compile()
    return nc


def _diag_masks():
    # dm2[k, v, h, q] = 1 if query q >= key k + v*128 else 0 (h duplicated)
    i = np.arange(QC)[None, :]
    j = np.arange(KB)[:, None]
    out = np.empty((KB, 4, 2, QC), np.float32)
    for v in range(4):
        mv = np.where(i >= j + v * KB, 1.0, 0.0).astype(np.float32)
        out[:, v, 0, :] = mv
        out[:, v, 1, :] = mv
    return out


def _prep_inputs(x, m, w_qkv, w_out):
    """Per-core input maps for SPMD dispatch (partition-contiguous packs)."""
    dmp = np.ascontiguousarray(_diag_masks()).astype(ml_dtypes.bfloat16)
    wq_full = w_qkv[:, 0:D]
    wk_full = w_qkv[:, D:2 * D]
    wv_full = w_qkv[:, 2 * D:3 * D]
    in_maps = []
    for c in range(8):
        b, q = c // 2, c % 2
        hsl = slice(4 * q * DH, (4 * q + 4) * DH)

        def pack_qk(w_full):
            # [dr, p, kc, o]: o spans the head pair (2*DH=128 cols)
            w2 = np.stack([w_full[:, (4 * q + 2 * p) * DH:(4 * q + 2 * p + 2) * DH]
                           for p in range(2)])            # [2, D, 128]
            return np.ascontiguousarray(
                w2.reshape(2, 4, 128, 128).transpose(2, 0, 1, 3))

        xpk = np.ascontiguousarray(
            x[b].T.reshape(4, 128, T).transpose(1, 0, 2))   # [128, 4, T]
        wvp = np.ascontiguousarray(
            wv_full[:, hsl].reshape(4, 128, 256).transpose(1, 0, 2))
        wop = np.ascontiguousarray(
            w_out[hsl, :].reshape(2, 128, D).transpose(1, 0, 2))
        mkp = np.ascontiguousarray(
            (m[b] != 0).astype(np.float32).reshape(16, 128).T)

        in_maps.append({
            "xp": xpk.astype(ml_dtypes.bfloat16),
            "wqp": pack_qk(wq_full).astype(ml_dtypes.bfloat16),
            "wkp": pack_qk(wk_full).astype(ml_dtypes.bfloat16),
            "wvp": wvp.astype(ml_dtypes.bfloat16),
            "wop": wop.astype(np.float32),
            "dmp": dmp,
            "mkp": mkp,
        })
    return in_maps


def _execute(inputs, trace=False):
    from concourse.bass_utils import run_bass_kernel_spmd

    if "nc" not in _CACHE:
        _CACHE["nc"] = _build_program()
    nc = _CACHE["nc"]

    x = np.asarray(inputs["x"], np.float32)
    m = np.asarray(inputs["m"], np.float32)
    w_qkv = np.asarray(inputs["w_qkv"], np.float32)
    w_out = np.asarray(inputs["w_out"], np.float32)
    b_out = np.asarray(inputs["b_out"], np.float32)

    in_maps = _prep_inputs(x, m, w_qkv, w_out)
    res = run_bass_kernel_spmd(nc, in_maps, core_ids=list(range(8)), trace=trace)

    y = np.empty((B, T, D), np.float32)
    for b in range(B):
        y[b] = res.results[2 * b]["y"] + res.results[2 * b + 1]["y"]
    y += b_out[None, None, :]
    y *= m[..., None]
    return y, res


def kernel(**inputs) -> np.ndarray:
    y, _ = _execute(inputs, trace=False)
    return y


# revision 58
# speedup vs baseline: 1.0286x; 1.0286x over previous
"""Trainium2 Bass kernel for nn_DiffusionModel_56822417326086.

Causal multi-head self-attention block:
    qkv = x @ w_qkv ; split into 8 heads of 64
    e = (q @ k^T) * DH^-0.5 ; causal + key-padding mask ; a = softmax(e)
    o = a @ v ; y = o @ w_out + b_out ; y *= m

Sharding (8 cores, zero collectives):
    core c -> batch b = c // 2, head-quad q = c % 2 (heads 4q..4q+3).
    Each core computes q/k/v for its 4 heads over its whole batch, full
    causal attention for those heads, and the partial output projection
    y_partial = o[heads] @ w_out[head rows].  Host sums the two partials
    per batch (linear unshard), adds b_out, applies the query-side mask.

v2 layout/scheduling notes (on top of the v1 design):
  - every input is host-packed so it loads as ONE partition-contiguous
    DMA (x as 4 column-chunks so the first QKV group starts early).
  - diagonal key blocks only compute/exp/accumulate the live query
    range (causal narrowing): scores, exp, mask-mul and A@V all shrink.
  - each (pair, qc) chunk's normalization + output projection is
    emitted inline so it overlaps the remaining attention; only the
    last chunk's chain trails the kernel.
  - softmax reciprocal uses reciprocal_approx_fast (~18 bits, 5x
    faster than the exact DVE reciprocal).
  - scores are computed TRANSPOSED: s[key, query] so the A@V
    contraction (over keys) has keys on the partition dim; softmax
    denominators come free as a 65th "ones" column of V; no
    max-subtraction (scores are O(1), exp is safe); matmuls are f32r /
    bf16; per-head operands sit at partition base 0/64 via the
    row-tiled PE array (tile_position).
"""

import numpy as np
import ml_dtypes
from contextlib import ExitStack

B, T, D, H = 4, 2048, 512, 8
DH = D // H
SCALE = DH ** -0.5
QC = 512           # query-chunk (free dim of score matmuls)
NQC = T // QC      # 4
KB = 128           # key-block (partition dim of score tiles)

_CACHE = {}


def _build_program():
    import concourse.mybir as mybir
    import concourse.tile as tile
    from concourse import bacc

    f32 = mybir.dt.float32
    f32r = mybir.dt.float32r
    bf16 = mybir.dt.bfloat16
    Exp = mybir.ActivationFunctionType.Exp

    nc = bacc.Bacc("TRN2", target_bir_lowering=False, debug=False)

    xp_d = nc.dram_tensor("xp", [128, 4, T], bf16, kind="ExternalInput").ap()
    wq_d = nc.dram_tensor("wqp", [128, 2, 4, 128], bf16, kind="ExternalInput").ap()
    wk_d = nc.dram_tensor("wkp", [128, 2, 4, 128], bf16, kind="ExternalInput").ap()
    wv_d = nc.dram_tensor("wvp", [128, 4, 256], bf16, kind="ExternalInput").ap()
    wo_d = nc.dram_tensor("wop", [128, 2, D], f32r, kind="ExternalInput").ap()
    dm_d = nc.dram_tensor("dmp", [128, 4, 2, 512], bf16, kind="ExternalInput").ap()
    mk_d = nc.dram_tensor("mkp", [128, 16], f32, kind="ExternalInput").ap()
    y_d = nc.dram_tensor("y", [T, D], f32, kind="ExternalOutput").ap()

    with tile.TileContext(nc) as tc, ExitStack() as ctx:
        consts = ctx.enter_context(tc.tile_pool(name="consts", bufs=1))
        work = ctx.enter_context(tc.tile_pool(name="work", bufs=2))
        ps_s = ctx.enter_context(tc.tile_pool(name="pss", bufs=2, space="PSUM"))
        ps_o = ctx.enter_context(tc.tile_pool(name="pso", bufs=1, space="PSUM"))
        ps_y = ctx.enter_context(tc.tile_pool(name="psy", bufs=2, space="PSUM"))

        # ---- persistent tiles ----------------------------------------------
        # packed q^T/k^T: partitions 0-63 = head A of pair, 64-127 = head B
        qT2 = consts.tile([128, 2, T], f32r)
        kT2 = consts.tile([128, 2, T], f32r)
        vsb = consts.tile([128, 16, 4, 65], bf16)
        wo = consts.tile([128, 2, D], f32r)
        mk = consts.tile([128, 16], f32)
        dm2 = consts.tile([128, 4, 2, 512], bf16)
        ones41 = consts.tile([128, 4, 1], f32)
        oUA = consts.tile([65, 2, T], f32)
        oUB = consts.tile([65, 2, T], f32)
        # pair-0 sums partition-parallel (rows 0-3, one batched recip, rec
        # row DMA'd to partition 0); pair-1 reciprocals computed in place at
        # partition 64 (straight from the oU sums row — legal DVE base) and
        # fed to the bc matmul via tile_position=(64, 0), no DMA hops.
        sums_stage = consts.tile([4, 1024], f32)    # row = qc (pair 0)
        recips = consts.tile([4, 1024], f32)
        recb = consts.tile([4, 1024], bf16)
        sums1 = consts.tile([1, 4, 1024], f32)      # [0, qc, :] (pair 1)
        recips1 = consts.tile([1, 4, 1024], f32)
        recb1 = consts.tile([1, 4, 1024], bf16)
        ones64 = consts.tile([1, 64], f32)
        ones64b = consts.tile([1, 64], bf16)
        oTn2 = consts.tile([128, 2, T], f32r)
        dummy = consts.tile([1, 1], f32)

        # ---- input DMAs (one big descriptor set each) + PE warmup ----------
        xT = consts.tile([128, 4, T], bf16)
        wq = consts.tile([128, 2, 4, 128], bf16)
        wk = consts.tile([128, 2, 4, 128], bf16)
        wv = consts.tile([128, 4, 256], bf16)

        # first QKV group needs wq/wk + x cols 0:512 -> issue those first on
        # the two fast HWDGE engines; the rest trickles in on gpsimd
        nc.scalar.dma_start(xT[:, :, 0:512], xp_d[:, :, 0:512])
        nc.sync.dma_start(wq[:], wq_d[:])
        nc.sync.dma_start(wk[:], wk_d[:])
        nc.scalar.dma_start(xT[:, :, 512:1024], xp_d[:, :, 512:1024])
        nc.sync.dma_start(xT[:, :, 1024:1536], xp_d[:, :, 1024:1536])
        nc.scalar.dma_start(xT[:, :, 1536:2048], xp_d[:, :, 1536:2048])
        nc.gpsimd.dma_start(wv[:], wv_d[:])
        nc.gpsimd.dma_start(wo[:], wo_d[:])
        nc.gpsimd.dma_start(mk[:], mk_d[:])
        nc.gpsimd.dma_start(dm2[:], dm_d[:])

        nc.vector.memset(ones41[:], 1.0)
        nc.vector.memset(ones64[:], 1.0)
        nc.vector.memset(dummy[:], 0.0)
        nc.vector.tensor_copy(ones64b[:], ones64[:])
        # preload the Exp table while DMAs are in flight
        nc.scalar.activation(dummy[:], dummy[:], Exp)
        warm = consts.tile([1, 512], bf16)
        nc.vector.memset(warm[:], 1.0)
        for _ in range(6):
            wps = ps_y.tile([64, 512], f32, tag="py")
            nc.tensor.matmul(wps[:], ones64b[:], warm[:], start=True, stop=True)

        # ---- q/k projection (emitted interleaved with pair-0 attention) ----
        def qk_group(nc, p, rc4):
            sl = slice(rc4 * 512, (rc4 + 1) * 512)
            psq = ps_s.tile([128, 512], f32, tag="s")
            psk = ps_y.tile([128, 512], f32, tag="py")
            for kc in range(4):
                nc.tensor.matmul(psq[:], wq[:, p, kc, :], xT[:, kc, sl],
                                 start=kc == 0, stop=kc == 3)
                nc.tensor.matmul(psk[:], wk[:, p, kc, :], xT[:, kc, sl],
                                 start=kc == 0, stop=kc == 3)
            nc.vector.tensor_copy(qT2[:, p, sl], psq[:])
            nc.vector.tensor_copy(kT2[:, p, sl], psk[:])

        def v_group(nc, rc):
            psv = ps_o.tile([128, 4, 64], f32, tag="oA" if rc % 2 == 0 else "oB")
            for kc in range(4):
                nc.tensor.matmul(psv[:], xT[:, kc, rc * 128:(rc + 1) * 128],
                                 wv[:, kc, :], start=kc == 0, stop=kc == 3)
            nc.vector.tensor_scalar_mul(vsb[:, rc, :, 0:64], psv[:],
                                        mk[:, rc:rc + 1])
            nc.vector.tensor_scalar_mul(vsb[:, rc, :, 64:65], ones41[:],
                                        mk[:, rc:rc + 1])

        # ---- attention with inline postludes --------------------------------
        def attn_chunk(nc, exp_pool, p, qc, inject=None):
            """Score+exp+mask+AV loop for one (pair, query-chunk).

            inject: {kb: [thunk, ...]} -- filler work (next chunk's qk/v
            projection groups) emitted inside the loop so the exp stream
            never starves at chunk boundaries."""
            nkb = 4 * (qc + 1)
            q0 = qc * QC
            oA = ps_o.tile([128, 512], f32, tag="oA")
            oB = ps_o.tile([128, 512], f32, tag="oB")
            avq = []

            def emit_av(item):
                kb, off, ex = item
                nc.tensor.matmul(oA[0:65, off:512], vsb[:, kb, 2 * p, :],
                                 ex[:, 0, off:512],
                                 start=kb == 0, stop=kb == nkb - 1)
                nc.tensor.matmul(oB[0:65, off:512], vsb[:, kb, 2 * p + 1, :],
                                 ex[:, 1, off:512],
                                 start=kb == 0, stop=kb == nkb - 1)

            for kb in range(nkb):
                kbrel = kb - (nkb - 4)
                off = max(0, kbrel) * KB
                ksl = slice(kb * KB, (kb + 1) * KB)
                qsl = slice(q0 + off, q0 + QC)
                sps = ps_s.tile([128, 2, 512], f32, tag="s")
                # row-tiled pair: K=64 each, concurrent in the array;
                # outputs land in DIFFERENT PSUM banks (same-bank
                # dual-write faults the exec unit)
                nc.tensor.matmul(sps[:, 0, off:512], kT2[0:64, p, ksl],
                                 qT2[0:64, p, qsl], start=True, stop=True,
                                 tile_position=(0, 0))
                nc.tensor.matmul(sps[:, 1, off:512], kT2[64:128, p, ksl],
                                 qT2[64:128, p, qsl], start=True, stop=True,
                                 tile_position=(64, 0))
                ex = exp_pool.tile([128, 2, 512], bf16, tag="exp")
                nc.scalar.activation(ex[:, :, off:512], sps[:, :, off:512],
                                     Exp, scale=SCALE)
                if kbrel >= 0:
                    nc.vector.tensor_mul(ex[:, :, off:512], ex[:, :, off:512],
                                         dm2[:, kbrel, :, off:512])
                avq.append((kb, off, ex))
                if len(avq) > 2:
                    emit_av(avq.pop(0))
                if inject and kb in inject:
                    for thunk in inject[kb]:
                        thunk()
            while avq:
                emit_av(avq.pop(0))
            return oA, oB

        def copies_and_sums(nc, p, qc, oA, oB, c0=0, cl=512, cast=True):
            qsl = slice(qc * QC + c0, qc * QC + c0 + cl)
            nc.vector.tensor_copy(oUA[0:65, p, qsl], oA[0:65, c0:c0 + cl])
            nc.vector.tensor_copy(oUB[0:65, p, qsl], oB[0:65, c0:c0 + cl])
            if p == 0:
                nc.gpsimd.dma_start(sums_stage[qc:qc + 1, 0:512],
                                    oUA[64:65, 0, qsl])
                nc.gpsimd.dma_start(sums_stage[qc:qc + 1, 512:1024],
                                    oUB[64:65, 0, qsl])
            else:
                sA = slice(2 * c0, 2 * c0 + cl)
                sB = slice(2 * c0 + cl, 2 * c0 + 2 * cl)
                nc.sync.dma_start(sums1[0:1, qc, sA], oUA[64:65, 1, qsl])
                nc.sync.dma_start(sums1[0:1, qc, sB], oUB[64:65, 1, qsl])
                sAB = slice(2 * c0, 2 * c0 + 2 * cl)
                nc.vector.reciprocal_approx_fast(recips1[0:1, qc, sAB],
                                                 sums1[0:1, qc, sAB])
                if cast:
                    nc.vector.tensor_copy(recb1[0:1, qc, sAB],
                                          recips1[0:1, qc, sAB])

        def norm_chain(nc, p, qc, c0=0, cl=512, raw_rec=False):
            """rec broadcast + normalize into oTn2 for one (pair, qc)."""
            qsl = slice(qc * QC + c0, qc * QC + c0 + cl)
            ones = ones64b[:]
            if p == 0:
                rec = work.tile([1, 1024], bf16, tag="rec")
                nc.sync.dma_start(rec[:], recb[qc:qc + 1, :])
                recA, recB = rec[0:1, 0:512], rec[0:1, 512:1024]
            elif raw_rec:
                # skip the bf16 cast on the tail chain; f32 moving operand
                recA = recips1[0:1, qc, 2 * c0:2 * c0 + cl]
                recB = recips1[0:1, qc, 2 * c0 + cl:2 * c0 + 2 * cl]
                ones = ones64[:]
            else:
                recA = recb1[0:1, qc, 2 * c0:2 * c0 + cl]
                recB = recb1[0:1, qc, 2 * c0 + cl:2 * c0 + 2 * cl]
            bcA = ps_y.tile([64, 512], f32, tag="py")
            nc.tensor.matmul(bcA[0:64, 0:cl], ones, recA,
                             start=True, stop=True)
            nc.vector.tensor_mul(oTn2[0:64, p, qsl], oUA[0:64, p, qsl],
                                 bcA[0:64, 0:cl])
            bcB = ps_y.tile([64, 512], f32, tag="py")
            nc.tensor.matmul(bcB[0:64, 0:cl], ones, recB,
                             start=True, stop=True)
            scrB = work.tile([64, 512], f32r, tag="scrB")
            nc.vector.tensor_mul(scrB[0:64, 0:cl], oUB[0:64, p, qsl],
                                 bcB[0:64, 0:cl])
            # partition shift 0-63 -> 64-127 (DVE lanes are partition-locked)
            nc.sync.dma_start(oTn2[64:128, p, qsl], scrB[0:64, 0:cl])

        def out_proj(nc, rcs, tail=False, act_copy=False):
            # gpsimd (Pool) DGE has multi-us start latency -- fine for hidden
            # mid-kernel stores, fatal for the tail ones.  Tail y stores go
            # on scalar so the sync ring stays clear for the shift DMAs.
            engs = [nc.scalar, nc.scalar] if tail else \
                [nc.sync, nc.gpsimd, nc.sync, nc.gpsimd]
            for i, rc in enumerate(rcs):
                rsl = slice(rc * 128, (rc + 1) * 128)
                psy = ps_y.tile([128, 512], f32, tag="py")
                for p in range(2):
                    nc.tensor.matmul(psy[:], oTn2[:, p, rsl], wo[:, p, :],
                                     start=p == 0, stop=p == 1)
                yt = work.tile([128, 512], f32, tag="ysb", bufs=4)
                if act_copy and i == 0:
                    nc.scalar.copy(yt[:], psy[:])
                else:
                    nc.vector.tensor_copy(yt[:], psy[:])
                engs[i].dma_start(y_d[rsl, :], yt[:])

        with tc.tile_pool(name="exp", bufs=6) as exp_pool:
            # pair 0: qk/v-projection groups interleaved chunk-by-chunk so
            # the exp stream (the attention pacer) starts as early as
            # possible; later chunks' qk/v groups are injected INSIDE the
            # kb loop (one group per iteration) so ACT never starves at
            # chunk boundaries
            for qc in range(NQC):
                qk_group(nc, 0, qc)
                if qc >= 1:
                    qk_group(nc, 1, qc - 1)
                for rc in range(4 * qc, 4 * qc + 4):
                    v_group(nc, rc)
                oA, oB = attn_chunk(nc, exp_pool, 0, qc)
                copies_and_sums(nc, 0, qc, oA, oB)
            qk_group(nc, 1, 3)

            # pair 1: attention with pair-0 normalization interleaved and
            # pair-1 postludes inline (chunk qc's full output emitted as
            # soon as its attention is done)
            for qc in range(NQC):
                oA, oB = attn_chunk(nc, exp_pool, 1, qc)
                if qc == 0:
                    nc.vector.reciprocal_approx_fast(recips[0:4, :],
                                                     sums_stage[0:4, :])
                    nc.vector.tensor_copy(recb[0:4, :], recips[0:4, :])
                if qc < NQC - 1:
                    copies_and_sums(nc, 1, qc, oA, oB)
                    norm_chain(nc, 0, qc)
                    norm_chain(nc, 1, qc)
                    out_proj(nc, range(4 * qc, 4 * qc + 4))
                    if qc == NQC - 2:
                        # pair-0's last-chunk normalization has no late deps;
                        # run it under the last chunk's attention
                        norm_chain(nc, 0, NQC - 1)
                else:
                    # last chunk: postlude in two pipelined 256-col halves so
                    # the tail chain shortens
                    copies_and_sums(nc, 1, qc, oA, oB, 0, 256, cast=False)
                    copies_and_sums(nc, 1, qc, oA, oB, 256, 256, cast=False)
                    norm_chain(nc, 1, qc, 0, 256, raw_rec=True)
                    out_proj(nc, [4 * qc, 4 * qc + 1], tail=True)
                    norm_chain(nc, 1, qc, 256, 256, raw_rec=True)
                    out_proj(nc, [4 * qc + 2, 4 * qc + 3], tail=True,
                             act_copy=True)

    nc.compile()
    return nc


def _diag_masks():
    # dm2[k, v, h, q] = 1 if query q >= key k + v*128 else 0 (h duplicated)
    i = np.arange(QC)[None, :]
    j = np.arange(KB)[:, None]
    out = np.empty((KB, 4, 2, QC), np.float32)
    for v in range(4):
        mv = np.where(i >= j + v * KB, 1.0, 0.0).astype(np.float32)
        out[:, v, 0, :] = mv
        out[:, v, 1, :] = mv
    return out


def _prep_inputs(x, m, w_qkv, w_out):
    """Per-core input maps for SPMD dispatch (partition-contiguous packs)."""
    dmp = np.ascontiguousarray(_diag_masks()).astype(ml_dtypes.bfloat16)
    wq_full = w_qkv[:, 0:D]
    wk_full = w_qkv[:, D:2 * D]
    wv_full = w_qkv[:, 2 * D:3 * D]
    in_maps = []
    for c in range(8):
        b, q = c // 2, c % 2
        hsl = slice(4 * q * DH, (4 * q + 4) * DH)

        def pack_qk(w_full):
            # [dr, p, kc, o]: o spans the head pair (2*DH=128 cols)
            w2 = np.stack([w_full[:, (4 * q + 2 * p) * DH:(4 * q + 2 * p + 2) * DH]
                           for p in range(2)])            # [2, D, 128]
            return np.ascontiguousarray(
                w2.reshape(2, 4, 128, 128).transpose(2, 0, 1, 3))

        xpk = np.ascontiguousarray(
            x[b].T.reshape(4, 128, T).transpose(1, 0, 2))   # [128, 4, T]
        wvp = np.ascontiguousarray(
            wv_full[:, hsl].reshape(4, 128, 256).transpose(1, 0, 2))
        wop = np.ascontiguousarray(
            w_out[hsl, :].reshape(2, 128, D).transpose(1, 0, 2))
        mkp = np.ascontiguousarray(
            (m[b] != 0).astype(np.float32).reshape(16, 128).T)

        in_maps.append({
            "xp": xpk.astype(ml_dtypes.bfloat16),
            "wqp": pack_qk(wq_full).astype(ml_dtypes.bfloat16),
            "wkp": pack_qk(wk_full).astype(ml_dtypes.bfloat16),
            "wvp": wvp.astype(ml_dtypes.bfloat16),
            "wop": wop.astype(np.float32),
            "dmp": dmp,
            "mkp": mkp,
        })
    return in_maps


def _execute(inputs, trace=False):
    from concourse.bass_utils import run_bass_kernel_spmd

    if "nc" not in _CACHE:
        _CACHE["nc"] = _build_program()
    nc = _CACHE["nc"]

    x = np.asarray(inputs["x"], np.float32)
    m = np.asarray(inputs["m"], np.float32)
    w_qkv = np.asarray(inputs["w_qkv"], np.float32)
    w_out = np.asarray(inputs["w_out"], np.float32)
    b_out = np.asarray(inputs["b_out"], np.float32)

    in_maps = _prep_inputs(x, m, w_qkv, w_out)
    res = run_bass_kernel_spmd(nc, in_maps, core_ids=list(range(8)), trace=trace)

    y = np.empty((B, T, D), np.float32)
    for b in range(B):
        y[b] = res.results[2 * b]["y"] + res.results[2 * b + 1]["y"]
    y += b_out[None, None, :]
    y *= m[..., None]
    return y, res


def kernel(**inputs) -> np.ndarray:
    y, _ = _execute(inputs, trace=False)
    return y
